# revision 37
# baseline (speedup 1.0000x reference)
"""CentroidInstanceLoss on 8 Trainium2 NeuronCores.

Strategy: shard by subbatch (B=8 subbatches -> 8 cores). The whole loss
decomposes per subbatch, so there are no cross-core collectives.

Per core, for its subbatch's point range [s, e):
  - Bulk: the 64-aligned inner range [ceil64(s), floor64(e)). Because
    labels[i] == i % 64 (spec fill: arange), the label of a bulk point is
    exactly its free-dim position j in a [128, 64, 16] chunk layout whose
    global base is 64-aligned. Segment sums over (label) then become plain
    partition reductions (ones-vector matmuls on the PE), with per-slot
    row-validity weights in the lhsT column.
  - Edges: the <=126 points outside the aligned range are processed with
    host-built one-hot matrices (one-hot matmuls on the PE).
  - Counts per (subbatch, label) are pure host arithmetic.

Perf notes (v2):
  - Points are staged to HBM as bf16 on the host (halves DMA bytes; the
    loss tolerance is 2e-2, bf16 compute error is ~1e-3).
  - All elementwise work runs on DVE at 2x mode (flat contiguous bf16
    APs) or on ACT; GpSimd does nothing but DMA triggers are on SyncE.
    (POOL and DVE share an SBUF port: concurrent use slows both.)
  - 1/||x|| is one ACT op (Abs_reciprocal_sqrt) whose input AP re-reads
    each sum-of-squares 16x, so the output is already expanded to
    per-element width -> the normalize multiply runs at DVE 2x.
  - Sum of squares over D=16 uses a TT add-tree (2x) instead of a 1x
    tensor_reduce.
  - Ops are fused over 4 slots (free dim 4096) to amortize dispatch.

If any structural assumption fails (labels != arange%64, unsorted subbatch,
empty segments, oversized subbatch), falls back to an exact numpy port of
the reference.
"""

import numpy as np

N = 2_000_000
D = 16
B = 8
L = 64
DELTA_V = 0.5
DELTA_D = 1.5

P = 128            # SBUF partitions
PPT = 64           # points per partition per slot
CHUNK = P * PPT    # 8192 points per slot
NSLOT = 32         # slots per core
SS = 4             # slots fused per op
NSUP = NSLOT // SS # superslots
PADPTS = NSLOT * CHUNK  # 262144 padded points per core
FD = PPT * D       # 1024 free elements per partition per slot
FDS = FD * SS      # 4096 free elements per superslot op

_PROGRAM_CACHE = {}


# ----------------------------------------------------------------------------
# numpy fallback (exact port of the reference; used only for off-spec inputs)
# ----------------------------------------------------------------------------
def _reference_numpy(outputs, labels, subbatch_indices):
    x = outputs.astype(np.float64)
    x = x / (np.linalg.norm(x, axis=1) + 1e-8)[:, None]
    seg = subbatch_indices.astype(np.int64) * L + labels.astype(np.int64)
    S = B * L
    counts = np.bincount(seg, minlength=S).astype(np.float64)
    sums = np.zeros((S, D), np.float64)
    np.add.at(sums, seg, x)
    mus = sums / counts[:, None]
    d1 = np.abs(mus[seg] - x).sum(axis=1)
    pull_pt = np.square(np.maximum(d1 - DELTA_V, 0.0))
    pull_seg = np.zeros((S,), np.float64)
    np.add.at(pull_seg, seg, pull_pt)
    M = L
    pull_b = (pull_seg / (M * counts)).reshape(B, L).sum(axis=1)
    mub = mus.reshape(B, L, D)
    dist = np.abs(mub[:, :, None, :] - mub[:, None, :, :]).sum(axis=-1)
    push = np.square(np.maximum(2.0 * DELTA_D - dist, 0.0))
    push = push * (1.0 - np.eye(L))
    push_b = push.sum(axis=(1, 2)) / (M * (M - 1))
    return np.float32(((pull_b + push_b) / B).sum())


# ----------------------------------------------------------------------------
# device program
# ----------------------------------------------------------------------------
def _build_program():
    import concourse.bacc as bacc
    import concourse.mybir as mybir
    import concourse.tile as tile

    f32 = mybir.dt.float32
    bf16 = mybir.dt.bfloat16
    AX = mybir.AxisListType
    OP = mybir.AluOpType
    AF = mybir.ActivationFunctionType

    nc = bacc.Bacc("TRN2", target_bir_lowering=False, debug=False)

    # xs is staged d-major per 64-point row: row r holds x[64r:64r+64, :].T
    # flattened, i.e. [d, j] with j (=label position) innermost.
    xs = nc.dram_tensor("xs", [NSLOT * P, FD], bf16, kind="ExternalInput").ap()
    pat = nc.dram_tensor("pat", [P, NSLOT], bf16, kind="ExternalInput").ap()
    ex = nc.dram_tensor("ex", [P, D], f32, kind="ExternalInput").ap()
    eoh = nc.dram_tensor("eoh", [P, L], bf16, kind="ExternalInput").ap()
    eohT = nc.dram_tensor("eohT", [L, P], bf16, kind="ExternalInput").ap()
    rc = nc.dram_tensor("rc", [D, L], f32, kind="ExternalInput").ap()
    rp = nc.dram_tensor("rp", [1, L], f32, kind="ExternalInput").ap()
    dm = nc.dram_tensor("dm", [L, L], bf16, kind="ExternalInput").ap()
    id16 = nc.dram_tensor("id16", [D, D], bf16, kind="ExternalInput").ap()
    out = nc.dram_tensor("out", [1, 1], f32, kind="ExternalOutput").ap()

    # superslot t, partition p, slot-in-superslot s, then (dim d, point j)
    xs_r = xs.rearrange("(t s p) f -> t p s f", t=NSUP, s=SS, p=P)

    with tile.TileContext(nc) as tc, nc.allow_low_precision(
            reason="bf16 intermediates are within the loss tolerance"):
        with (
            tc.tile_pool(name="const", bufs=1) as const,
            tc.tile_pool(name="xbp", bufs=4) as xbp,
            tc.tile_pool(name="sqp", bufs=2) as sqp,
            tc.tile_pool(name="adp", bufs=2) as adp,
            tc.tile_pool(name="trp", bufs=2) as trp,
            tc.tile_pool(name="ssp", bufs=2) as ssp,
            tc.tile_pool(name="rxp", bufs=2) as rxp,
            tc.tile_pool(name="xhp", bufs=NSUP) as xhp,
            tc.tile_pool(name="dfp", bufs=2) as dfp,
            tc.tile_pool(name="d1p", bufs=2) as d1p,
            tc.tile_pool(name="ppp", bufs=2) as ppp,
            tc.tile_pool(name="tmp", bufs=2) as tmp,
            tc.tile_pool(name="fin", bufs=1) as fin,
            tc.tile_pool(name="ps", bufs=1, space="PSUM") as ps,
            tc.tile_pool(name="ps2", bufs=2, space="PSUM") as ps2,
            tc.tile_pool(name="psp", bufs=1, space="PSUM") as psp,
        ):
            # ---- constants ----
            pat_sb = const.tile([P, NSLOT], bf16, tag="pat")
            nc.gpsimd.dma_start(out=pat_sb, in_=pat)
            ex_sb = const.tile([P, D], f32, tag="ex")
            nc.gpsimd.dma_start(out=ex_sb, in_=ex)
            eoh_sb = const.tile([P, L], bf16, tag="eoh")
            nc.gpsimd.dma_start(out=eoh_sb, in_=eoh)
            eohT_sb = const.tile([L, P], bf16, tag="eohT")
            nc.gpsimd.dma_start(out=eohT_sb, in_=eohT)
            rc_sb = const.tile([D, L], f32, tag="rc")
            nc.gpsimd.dma_start(out=rc_sb, in_=rc)
            rp_sb = const.tile([1, L], f32, tag="rp")
            nc.gpsimd.dma_start(out=rp_sb, in_=rp)
            dm_sb = const.tile([L, L], bf16, tag="dm")
            nc.gpsimd.dma_start(out=dm_sb, in_=dm)
            id16_sb = const.tile([D, D], bf16, tag="id16")
            nc.gpsimd.dma_start(out=id16_sb, in_=id16)
            onescol = const.tile([1, P], bf16, tag="onescol")
            nc.vector.memset(onescol, 1.0)
            ones64 = const.tile([L, 1], f32, tag="ones64")
            nc.vector.memset(ones64, 1.0)
            negdv = const.tile([P, 1], f32, tag="negdv")
            nc.vector.memset(negdv, -DELTA_V)
            twodd = const.tile([P, 1], f32, tag="twodd")
            nc.vector.memset(twodd, 2.0 * DELTA_D)

            xh_tiles = []

            # ---- pass 1: normalize points, accumulate per-label sums ----
            # Software-pipelined: rec(t-1) is issued after sq(t) in the ACT
            # stream and mult(t-1) after tree(t) in the DVE stream, so
            # neither engine head-of-line blocks on the other.
            sums_ps = ps.tile([1, FD], f32, tag="big")
            xb_tiles = [None] * NSUP
            ss_tiles = [None] * NSUP
            rx_tiles = [None] * NSUP

            def p1_rec(t):
                rx_t = rxp.tile([P, SS, PPT], bf16, tag="rx")
                rx_tiles[t] = rx_t
                nc.scalar.activation(out=rx_t, in_=ss_tiles[t],
                                     func=AF.Abs_reciprocal_sqrt)

            def p1_mult_mm(t):
                xb_t = xb_tiles[t]
                xh_t = xhp.tile([P, FDS], bf16, tag="xh")
                xh_tiles.append(xh_t)
                nc.vector.tensor_tensor(
                    out=xh_t.rearrange("p (s d j) -> p s d j", s=SS, d=D),
                    in0=xb_t.rearrange("p (s d j) -> p s d j", s=SS, d=D),
                    in1=rx_tiles[t].unsqueeze(2).to_broadcast([P, SS, D, PPT]),
                    op=OP.mult)
                for s in range(SS):
                    for h in range(2):
                        nc.tensor.matmul(
                            out=sums_ps[:, h * 512:(h + 1) * 512],
                            lhsT=pat_sb[:, t * SS + s:t * SS + s + 1],
                            rhs=xh_t[:, s * FD + h * 512:s * FD + (h + 1) * 512],
                            start=(t == 0 and s == 0),
                            stop=(t == NSUP - 1 and s == SS - 1))

            for t in range(NSUP):
                xb_t = xbp.tile([P, FDS], bf16, tag="xb")
                xb_tiles[t] = xb_t
                dma_eng = nc.sync if t % 2 == 0 else nc.gpsimd
                dma_eng.dma_start(
                    out=xb_t.rearrange("p (s f) -> p s f", s=SS), in_=xs_r[t])
                # squares on ACT (keeps DVE free)
                sq_t = sqp.tile([P, FDS], bf16, tag="sq")
                nc.scalar.activation(out=sq_t, in_=xb_t, func=AF.Square)
                if t >= 1:
                    p1_rec(t - 1)
                # sum over D=16 via a 2x TT add-tree on DVE; d-major layout
                # makes every tree operand a contiguous block.
                sq3 = sq_t.rearrange("p (s f) -> p s f", s=SS)
                t1 = trp.tile([P, SS, 512], bf16, tag="t1")
                nc.vector.tensor_tensor(out=t1, in0=sq3[:, :, 0:512],
                                        in1=sq3[:, :, 512:1024], op=OP.add)
                t2 = trp.tile([P, SS, 256], bf16, tag="t2")
                nc.vector.tensor_tensor(out=t2, in0=t1[:, :, 0:256],
                                        in1=t1[:, :, 256:512], op=OP.add)
                t3 = trp.tile([P, SS, 128], bf16, tag="t3")
                nc.vector.tensor_tensor(out=t3, in0=t2[:, :, 0:128],
                                        in1=t2[:, :, 128:256], op=OP.add)
                ss_t = ssp.tile([P, SS, PPT], f32, tag="ss")
                ss_tiles[t] = ss_t
                nc.vector.tensor_tensor(out=ss_t, in0=t3[:, :, 0:PPT],
                                        in1=t3[:, :, PPT:128], op=OP.add)
                if t >= 1:
                    p1_mult_mm(t - 1)
            p1_rec(NSUP - 1)
            p1_mult_mm(NSUP - 1)

            # ---- edge points: normalize + one-hot sums ----
            exsq = tmp.tile([P, D], f32, tag="exsq")
            nc.vector.tensor_tensor(out=exsq, in0=ex_sb, in1=ex_sb, op=OP.mult)
            ess = tmp.tile([P, 1], f32, tag="ess")
            nc.vector.tensor_reduce(out=ess, in_=exsq, axis=AX.X, op=OP.add)
            erc = tmp.tile([P, 1], f32, tag="erc")
            nc.scalar.activation(out=erc, in_=ess, func=AF.Abs_reciprocal_sqrt)
            exh = fin.tile([P, D], bf16, tag="exh")
            nc.vector.tensor_scalar_mul(out=exh, in0=ex_sb, scalar1=erc)
            # transposed edge sums: out[d, l] = sum_p exh[p, d] * eoh[p, l]
            esums_ps = ps2.tile([D, L], f32, tag="small")
            nc.tensor.matmul(out=esums_ps, lhsT=exh, rhs=eoh_sb,
                             start=True, stop=True)

            # ---- centroids (in [D, L] orientation, matching d-major rows) ----
            sums_row = fin.tile([1, FD], f32, tag="sums_row")
            nc.vector.tensor_copy(out=sums_row, in_=sums_ps)
            sumsMatT = fin.tile([D, L], f32, tag="sumsMatT")
            nc.sync.dma_start(
                out=sumsMatT, in_=sums_row.rearrange("a (d l) -> a d l", d=D))
            esums_sb = fin.tile([D, L], f32, tag="esums_sb")
            nc.scalar.copy(out=esums_sb, in_=esums_ps)
            totMatT = fin.tile([D, L], f32, tag="totMatT")
            nc.vector.tensor_tensor(out=totMatT, in0=sumsMatT, in1=esums_sb,
                                    op=OP.add)
            muMatT = fin.tile([D, L], bf16, tag="muMatT")
            nc.vector.tensor_tensor(out=muMatT, in0=totMatT, in1=rc_sb,
                                    op=OP.mult)
            muT_ps = ps2.tile([L, D], f32, tag="small")
            nc.tensor.matmul(out=muT_ps, lhsT=muMatT, rhs=id16_sb,
                             start=True, stop=True)
            muMatb = fin.tile([L, D], bf16, tag="muMatb")
            nc.vector.tensor_copy(out=muMatb, in_=muT_ps)
            muRowb = fin.tile([1, FD], bf16, tag="muRowb")
            nc.sync.dma_start(
                out=muRowb.rearrange("a (d l) -> a d l", d=D), in_=muMatT)
            mubc_ps = ps.tile([P, FD], f32, tag="mubc")
            for h in range(2):
                nc.tensor.matmul(
                    out=mubc_ps[:, h * 512:(h + 1) * 512],
                    lhsT=onescol,
                    rhs=muRowb[:, h * 512:(h + 1) * 512],
                    start=True, stop=True)
            muExp = fin.tile([P, FD], bf16, tag="muExp")
            nc.vector.tensor_copy(out=muExp, in_=mubc_ps)

            # ---- pass 2: pull term ----
            pull_ps = psp.tile([1, L], f32, tag="pull")
            ad_tiles = [None] * NSUP

            def p2_tail(t):
                ad3 = ad_tiles[t].rearrange("p (s f) -> p s f", s=SS)
                u1 = trp.tile([P, SS, 512], bf16, tag="u1")
                nc.vector.tensor_tensor(out=u1, in0=ad3[:, :, 0:512],
                                        in1=ad3[:, :, 512:1024], op=OP.add)
                u2 = trp.tile([P, SS, 256], bf16, tag="u2")
                nc.vector.tensor_tensor(out=u2, in0=u1[:, :, 0:256],
                                        in1=u1[:, :, 256:512], op=OP.add)
                u3 = trp.tile([P, SS, 128], bf16, tag="u3")
                nc.vector.tensor_tensor(out=u3, in0=u2[:, :, 0:128],
                                        in1=u2[:, :, 128:256], op=OP.add)
                d1_t = d1p.tile([P, SS, PPT], f32, tag="d1")
                nc.vector.tensor_tensor(out=d1_t, in0=u3[:, :, 0:PPT],
                                        in1=u3[:, :, PPT:128], op=OP.add)
                pp_t = ppp.tile([P, SS * PPT], bf16, tag="pp")
                nc.scalar.activation(out=pp_t,
                                     in_=d1_t.rearrange("p s j -> p (s j)"),
                                     func=AF.Square, bias=negdv)
                for s in range(SS):
                    nc.tensor.matmul(
                        out=pull_ps,
                        lhsT=pat_sb[:, t * SS + s:t * SS + s + 1],
                        rhs=pp_t[:, s * PPT:(s + 1) * PPT],
                        start=(t == 0 and s == 0),
                        stop=(t == NSUP - 1 and s == SS - 1))

            for t in range(NSUP):
                xh_t = xh_tiles[t]
                diff_t = dfp.tile([P, FDS], bf16, tag="diff")
                nc.vector.tensor_tensor(
                    out=diff_t.rearrange("p (s f) -> p s f", s=SS),
                    in0=xh_t.rearrange("p (s f) -> p s f", s=SS),
                    in1=muExp.unsqueeze(1).to_broadcast([P, SS, FD]),
                    op=OP.subtract)
                # |diff| on ACT, then sum over D via contiguous 2x TT tree
                # (a strided tensor_reduce over d-major costs 1.7ns/elem)
                ad_t = adp.tile([P, FDS], bf16, tag="ad")
                ad_tiles[t] = ad_t
                nc.scalar.activation(out=ad_t, in_=diff_t, func=AF.Abs)
                p2_tail(t)

            # ---- edge pull ----
            medge_ps = ps2.tile([P, D], f32, tag="small")
            nc.tensor.matmul(out=medge_ps, lhsT=eohT_sb, rhs=muMatb,
                             start=True, stop=True)
            ediff = tmp.tile([P, D], bf16, tag="ediff")
            nc.vector.tensor_tensor(out=ediff, in0=exh, in1=medge_ps,
                                    op=OP.subtract)
            ed1 = tmp.tile([P, 1], f32, tag="ed1")
            nc.vector.tensor_reduce(out=ed1, in_=ediff, axis=AX.X, op=OP.add,
                                    apply_absolute_value=True)
            erl = tmp.tile([P, 1], bf16, tag="erl")
            nc.scalar.activation(out=erl, in_=ed1, func=AF.Relu, bias=negdv)
            epp = tmp.tile([P, 1], bf16, tag="epp")
            nc.vector.tensor_tensor(out=epp, in0=erl, in1=erl, op=OP.mult)
            pull2_ps = ps2.tile([1, L], f32, tag="small")
            nc.tensor.matmul(out=pull2_ps, lhsT=epp, rhs=eoh_sb,
                             start=True, stop=True)

            # ---- push term (pairwise centroid distances) ----
            diffP = fin.tile([L, L, D], bf16, tag="diffP")
            nc.vector.tensor_tensor(
                out=diffP,
                in0=muExp[:L, :].rearrange("l (d m) -> l m d", d=D),
                in1=muMatb.unsqueeze(1).to_broadcast([L, L, D]),
                op=OP.subtract)
            distP = fin.tile([L, L], f32, tag="distP")
            nc.vector.tensor_reduce(out=distP, in_=diffP, axis=AX.X, op=OP.add,
                                    apply_absolute_value=True)
            hingeP = fin.tile([L, L], f32, tag="hingeP")
            nc.scalar.activation(out=hingeP, in_=distP, func=AF.Relu,
                                 bias=twodd[:L], scale=-1.0)
            hm = fin.tile([L, L], bf16, tag="hm")
            nc.vector.tensor_tensor(out=hm, in0=hingeP, in1=dm_sb, op=OP.mult)
            psq = fin.tile([L, L], bf16, tag="psq")
            nc.vector.tensor_tensor(out=psq, in0=hm, in1=hm, op=OP.mult)
            pushrow = fin.tile([L, 1], f32, tag="pushrow")
            nc.vector.tensor_reduce(out=pushrow, in_=psq, axis=AX.X, op=OP.add)
            push_ps = ps2.tile([1, 1], f32, tag="small")
            nc.tensor.matmul(out=push_ps, lhsT=pushrow, rhs=ones64,
                             start=True, stop=True)

            # ---- finalize pull ----
            pull_row = fin.tile([1, L], f32, tag="pull_row")
            nc.vector.tensor_copy(out=pull_row, in_=pull_ps)
            pull_tot = fin.tile([1, L], f32, tag="pull_tot")
            nc.vector.tensor_tensor(out=pull_tot, in0=pull_row, in1=pull2_ps,
                                    op=OP.add)
            pullw = fin.tile([1, L], f32, tag="pullw")
            nc.vector.tensor_tensor(out=pullw, in0=pull_tot, in1=rp_sb,
                                    op=OP.mult)
            pullb = fin.tile([1, 1], f32, tag="pullb")
            nc.vector.tensor_reduce(out=pullb, in_=pullw, axis=AX.X, op=OP.add)

            # ---- total ----
            t1f = fin.tile([1, 1], f32, tag="t1f")
            nc.scalar.mul(t1f, push_ps, 1.0 / (L * (L - 1)))
            t2f = fin.tile([1, 1], f32, tag="t2f")
            nc.scalar.add(t2f, t1f, pullb)
            t3f = fin.tile([1, 1], f32, tag="t3f")
            nc.scalar.mul(t3f, t2f, 1.0 / B)
            nc.sync.dma_start(out=out, in_=t3f)

    nc.compile()
    return nc


def _get_program():
    if "nc" not in _PROGRAM_CACHE:
        _PROGRAM_CACHE["nc"] = _build_program()
    return _PROGRAM_CACHE["nc"]


# ----------------------------------------------------------------------------
# host orchestration
# ----------------------------------------------------------------------------
def _prep_core_inputs(x, lab, bounds, b):
    import ml_dtypes
    bf = ml_dtypes.bfloat16

    s, e = int(bounds[b]), int(bounds[b + 1])
    lo = -((-s) // 64) * 64
    hi = (e // 64) * 64
    if hi < lo:
        lo = hi = s  # tiny subbatch: no aligned bulk, everything is an edge
    bulk = hi - lo

    # d-major staging: row r holds x[64r:64r+64, :].T flattened
    xs_pad = np.ones((NSLOT * P, FD), bf)
    if bulk > 0:
        rows_bulk = bulk // 64
        xs_pad[:rows_bulk] = (
            x[lo:hi].reshape(rows_bulk, 64, D).transpose(0, 2, 1)
            .reshape(rows_bulk, FD).astype(bf))

    # pat[partition, slot]: partition p of slot sl covers the 64-point row
    # number sl*128 + p of the bulk; weight 1 iff that row is fully real.
    rows_real = bulk // 64
    row_idx = np.arange(NSLOT * P).reshape(NSLOT, P).T  # [P, NSLOT]
    pat = (row_idx < rows_real).astype(np.float32)

    eidx = np.concatenate([np.arange(s, lo), np.arange(hi, e)])
    ne = len(eidx)
    assert ne <= P
    ex_pad = np.ones((P, D), np.float32)
    eoh = np.zeros((P, L), np.float32)
    if ne > 0:
        ex_pad[:ne] = x[eidx]
        eoh[np.arange(ne), lab[eidx]] = 1.0

    n = e - s
    base = s % 64
    ls = np.arange(L)
    cnt = (n // 64) + (((ls - base) % 64) < (n % 64)).astype(np.int64)

    return {
        "xs": xs_pad,
        "pat": pat.astype(bf),
        "ex": ex_pad,
        "eoh": eoh.astype(bf),
        "eohT": eoh.T.astype(bf).copy(),
        "rc": np.repeat((1.0 / cnt.astype(np.float64))
                        .astype(np.float32)[None, :], D, axis=0),
        "rp": (1.0 / (L * cnt.astype(np.float64))).astype(np.float32)[None, :],
        "dm": (1.0 - np.eye(L, dtype=np.float32)).astype(bf),
        "id16": np.eye(D, dtype=np.float32).astype(bf),
    }


def _check_fast_path(x, lab, sub):
    if x.shape != (N, D):
        return False
    if lab.shape != (N,) or sub.shape != (N,):
        return False
    if not np.array_equal(lab, np.arange(N, dtype=np.int64) % L):
        return False
    if sub.min() < 0 or sub.max() >= B:
        return False
    if np.any(sub[1:] < sub[:-1]):
        return False
    return True


def kernel(outputs, labels, subbatch_indices):
    x = np.asarray(outputs, dtype=np.float32)
    lab = np.asarray(labels).astype(np.int64)
    sub = np.asarray(subbatch_indices).astype(np.int64)

    if not _check_fast_path(x, lab, sub):
        return _reference_numpy(x, lab, sub)

    bounds = np.searchsorted(sub, np.arange(B + 1), side="left")
    sizes = np.diff(bounds)
    if sizes.min() == 0 or sizes.max() > PADPTS + 126:
        return _reference_numpy(x, lab, sub)
    for b in range(B):
        s, e = int(bounds[b]), int(bounds[b + 1])
        lo = -((-s) // 64) * 64
        hi = (e // 64) * 64
        if hi - lo > PADPTS or (e - s) - max(hi - lo, 0) > P:
            return _reference_numpy(x, lab, sub)
        n, base = e - s, s % 64
        cnt = (n // 64) + (((np.arange(L) - base) % 64) < (n % 64))
        if cnt.min() <= 0:
            return _reference_numpy(x, lab, sub)

    from concourse import bass_utils

    nc = _get_program()
    in_maps = [_prep_core_inputs(x, lab, bounds, b) for b in range(B)]
    res = bass_utils.run_bass_kernel_spmd(nc, in_maps, list(range(B)))
    _PROGRAM_CACHE["last_results"] = res
    total = np.float32(0.0)
    for b in range(B):
        total += np.float32(res.results[b]["out"][0, 0])
    return np.float32(total)


if __name__ == "__main__":
    import reference
    inputs = {k: np.asarray(v) for k, v in reference.setup_inputs().items()}
    got = kernel(**inputs)
    print("kernel:", got)


# revision 38
# speedup vs baseline: 1.0101x; 1.0101x over previous
"""CentroidInstanceLoss on 8 Trainium2 NeuronCores.

Strategy: shard by subbatch (B=8 subbatches -> 8 cores). The whole loss
decomposes per subbatch, so there are no cross-core collectives.

Per core, for its subbatch's point range [s, e):
  - Bulk: the 64-aligned inner range [ceil64(s), floor64(e)). Because
    labels[i] == i % 64 (spec fill: arange), the label of a bulk point is
    exactly its free-dim position j in a [128, 64, 16] chunk layout whose
    global base is 64-aligned. Segment sums over (label) then become plain
    partition reductions (ones-vector matmuls on the PE), with per-slot
    row-validity weights in the lhsT column.
  - Edges: the <=126 points outside the aligned range are processed with
    host-built one-hot matrices (one-hot matmuls on the PE).
  - Counts per (subbatch, label) are pure host arithmetic.

Perf notes (v2):
  - Points are staged to HBM as bf16 on the host (halves DMA bytes; the
    loss tolerance is 2e-2, bf16 compute error is ~1e-3).
  - All elementwise work runs on DVE at 2x mode (flat contiguous bf16
    APs) or on ACT; GpSimd does nothing but DMA triggers are on SyncE.
    (POOL and DVE share an SBUF port: concurrent use slows both.)
  - 1/||x|| is one ACT op (Abs_reciprocal_sqrt) whose input AP re-reads
    each sum-of-squares 16x, so the output is already expanded to
    per-element width -> the normalize multiply runs at DVE 2x.
  - Sum of squares over D=16 uses a TT add-tree (2x) instead of a 1x
    tensor_reduce.
  - Ops are fused over 4 slots (free dim 4096) to amortize dispatch.

If any structural assumption fails (labels != arange%64, unsorted subbatch,
empty segments, oversized subbatch), falls back to an exact numpy port of
the reference.
"""

import numpy as np

N = 2_000_000
D = 16
B = 8
L = 64
DELTA_V = 0.5
DELTA_D = 1.5

P = 128            # SBUF partitions
PPT = 64           # points per partition per slot
CHUNK = P * PPT    # 8192 points per slot
NSLOT = 32         # slots per core
SS = 4             # slots fused per op
NSUP = NSLOT // SS # superslots
PADPTS = NSLOT * CHUNK  # 262144 padded points per core
FD = PPT * D       # 1024 free elements per partition per slot
FDS = FD * SS      # 4096 free elements per superslot op

_PROGRAM_CACHE = {}


# ----------------------------------------------------------------------------
# numpy fallback (exact port of the reference; used only for off-spec inputs)
# ----------------------------------------------------------------------------
def _reference_numpy(outputs, labels, subbatch_indices):
    x = outputs.astype(np.float64)
    x = x / (np.linalg.norm(x, axis=1) + 1e-8)[:, None]
    seg = subbatch_indices.astype(np.int64) * L + labels.astype(np.int64)
    S = B * L
    counts = np.bincount(seg, minlength=S).astype(np.float64)
    sums = np.zeros((S, D), np.float64)
    np.add.at(sums, seg, x)
    mus = sums / counts[:, None]
    d1 = np.abs(mus[seg] - x).sum(axis=1)
    pull_pt = np.square(np.maximum(d1 - DELTA_V, 0.0))
    pull_seg = np.zeros((S,), np.float64)
    np.add.at(pull_seg, seg, pull_pt)
    M = L
    pull_b = (pull_seg / (M * counts)).reshape(B, L).sum(axis=1)
    mub = mus.reshape(B, L, D)
    dist = np.abs(mub[:, :, None, :] - mub[:, None, :, :]).sum(axis=-1)
    push = np.square(np.maximum(2.0 * DELTA_D - dist, 0.0))
    push = push * (1.0 - np.eye(L))
    push_b = push.sum(axis=(1, 2)) / (M * (M - 1))
    return np.float32(((pull_b + push_b) / B).sum())


# ----------------------------------------------------------------------------
# device program
# ----------------------------------------------------------------------------
def _build_program():
    import concourse.bacc as bacc
    import concourse.mybir as mybir
    import concourse.tile as tile

    f32 = mybir.dt.float32
    bf16 = mybir.dt.bfloat16
    AX = mybir.AxisListType
    OP = mybir.AluOpType
    AF = mybir.ActivationFunctionType

    nc = bacc.Bacc("TRN2", target_bir_lowering=False, debug=False)

    # xs is staged d-major per 64-point row: row r holds x[64r:64r+64, :].T
    # flattened, i.e. [d, j] with j (=label position) innermost.
    xs = nc.dram_tensor("xs", [NSLOT * P, FD], bf16, kind="ExternalInput").ap()
    pat = nc.dram_tensor("pat", [P, NSLOT], bf16, kind="ExternalInput").ap()
    ex = nc.dram_tensor("ex", [P, D], f32, kind="ExternalInput").ap()
    eoh = nc.dram_tensor("eoh", [P, L], bf16, kind="ExternalInput").ap()
    eohT = nc.dram_tensor("eohT", [L, P], bf16, kind="ExternalInput").ap()
    rc = nc.dram_tensor("rc", [D, L], f32, kind="ExternalInput").ap()
    rp = nc.dram_tensor("rp", [1, L], f32, kind="ExternalInput").ap()
    dm = nc.dram_tensor("dm", [L, L], bf16, kind="ExternalInput").ap()
    id16 = nc.dram_tensor("id16", [D, D], bf16, kind="ExternalInput").ap()
    out = nc.dram_tensor("out", [1, 1], f32, kind="ExternalOutput").ap()

    # superslot t, partition p, slot-in-superslot s, then (dim d, point j)
    xs_r = xs.rearrange("(t s p) f -> t p s f", t=NSUP, s=SS, p=P)

    with tile.TileContext(nc) as tc, nc.allow_low_precision(
            reason="bf16 intermediates are within the loss tolerance"):
        with (
            tc.tile_pool(name="const", bufs=1) as const,
            tc.tile_pool(name="xbp", bufs=4) as xbp,
            tc.tile_pool(name="sqp", bufs=3) as sqp,
            tc.tile_pool(name="adp", bufs=2) as adp,
            tc.tile_pool(name="trp", bufs=2) as trp,
            tc.tile_pool(name="ssp", bufs=2) as ssp,
            tc.tile_pool(name="rxp", bufs=2) as rxp,
            tc.tile_pool(name="xhp", bufs=NSUP) as xhp,
            tc.tile_pool(name="dfp", bufs=2) as dfp,
            tc.tile_pool(name="d1p", bufs=2) as d1p,
            tc.tile_pool(name="ppp", bufs=2) as ppp,
            tc.tile_pool(name="tmp", bufs=2) as tmp,
            tc.tile_pool(name="fin", bufs=1) as fin,
            tc.tile_pool(name="ps", bufs=1, space="PSUM") as ps,
            tc.tile_pool(name="ps2", bufs=2, space="PSUM") as ps2,
            tc.tile_pool(name="psp", bufs=1, space="PSUM") as psp,
        ):
            # ---- constants ----
            pat_sb = const.tile([P, NSLOT], bf16, tag="pat")
            nc.gpsimd.dma_start(out=pat_sb, in_=pat)
            ex_sb = const.tile([P, D], f32, tag="ex")
            nc.gpsimd.dma_start(out=ex_sb, in_=ex)
            eoh_sb = const.tile([P, L], bf16, tag="eoh")
            nc.gpsimd.dma_start(out=eoh_sb, in_=eoh)
            eohT_sb = const.tile([L, P], bf16, tag="eohT")
            nc.gpsimd.dma_start(out=eohT_sb, in_=eohT)
            rc_sb = const.tile([D, L], f32, tag="rc")
            nc.gpsimd.dma_start(out=rc_sb, in_=rc)
            rp_sb = const.tile([1, L], f32, tag="rp")
            nc.gpsimd.dma_start(out=rp_sb, in_=rp)
            dm_sb = const.tile([L, L], bf16, tag="dm")
            nc.gpsimd.dma_start(out=dm_sb, in_=dm)
            id16_sb = const.tile([D, D], bf16, tag="id16")
            nc.gpsimd.dma_start(out=id16_sb, in_=id16)
            onescol = const.tile([1, P], bf16, tag="onescol")
            nc.vector.memset(onescol, 1.0)
            ones64 = const.tile([L, 1], f32, tag="ones64")
            nc.vector.memset(ones64, 1.0)
            negdv = const.tile([P, 1], f32, tag="negdv")
            nc.vector.memset(negdv, -DELTA_V)
            twodd = const.tile([P, 1], f32, tag="twodd")
            nc.vector.memset(twodd, 2.0 * DELTA_D)
            warm = const.tile([1, 1], f32, tag="warm")
            nc.vector.memset(warm, 1.0)
            warm2 = const.tile([1, 1], f32, tag="warm2")
            nc.scalar.activation(out=warm2, in_=warm, func=AF.Square)
            warm3 = const.tile([1, 1], f32, tag="warm3")
            nc.scalar.activation(out=warm3, in_=warm2,
                                 func=AF.Abs_reciprocal_sqrt)

            xh_tiles = []

            # ---- pass 1: normalize points, accumulate per-label sums ----
            # Software-pipelined: rec(t-1) is issued after sq(t) in the ACT
            # stream and mult(t-1) after tree(t) in the DVE stream, so
            # neither engine head-of-line blocks on the other.
            sums_ps = ps.tile([1, FD], f32, tag="big")
            xb_tiles = [None] * NSUP
            ss_tiles = [None] * NSUP
            rx_tiles = [None] * NSUP

            def p1_rec(t):
                rx_t = rxp.tile([P, SS, PPT], bf16, tag="rx")
                rx_tiles[t] = rx_t
                nc.scalar.activation(out=rx_t, in_=ss_tiles[t],
                                     func=AF.Abs_reciprocal_sqrt)

            def p1_mult_mm(t):
                xb_t = xb_tiles[t]
                xh_t = xhp.tile([P, FDS], bf16, tag="xh")
                xh_tiles.append(xh_t)
                nc.vector.tensor_tensor(
                    out=xh_t.rearrange("p (s d j) -> p s d j", s=SS, d=D),
                    in0=xb_t.rearrange("p (s d j) -> p s d j", s=SS, d=D),
                    in1=rx_tiles[t].unsqueeze(2).to_broadcast([P, SS, D, PPT]),
                    op=OP.mult)
                for s in range(SS):
                    for h in range(2):
                        nc.tensor.matmul(
                            out=sums_ps[:, h * 512:(h + 1) * 512],
                            lhsT=pat_sb[:, t * SS + s:t * SS + s + 1],
                            rhs=xh_t[:, s * FD + h * 512:s * FD + (h + 1) * 512],
                            start=(t == 0 and s == 0),
                            stop=(t == NSUP - 1 and s == SS - 1))

            for t in range(NSUP):
                xb_t = xbp.tile([P, FDS], bf16, tag="xb")
                xb_tiles[t] = xb_t
                dma_eng = nc.sync if t % 2 == 0 else nc.gpsimd
                dma_eng.dma_start(
                    out=xb_t.rearrange("p (s f) -> p s f", s=SS), in_=xs_r[t])
                # squares on ACT (keeps DVE free)
                sq_t = sqp.tile([P, FDS], bf16, tag="sq")
                nc.scalar.activation(out=sq_t, in_=xb_t, func=AF.Square)
                if t >= 1:
                    p1_rec(t - 1)
                # sum over D=16 via a 2x TT add-tree on DVE; d-major layout
                # makes every tree operand a contiguous block.
                sq3 = sq_t.rearrange("p (s f) -> p s f", s=SS)
                t1 = trp.tile([P, SS, 512], bf16, tag="t1")
                nc.vector.tensor_tensor(out=t1, in0=sq3[:, :, 0:512],
                                        in1=sq3[:, :, 512:1024], op=OP.add)
                t2 = trp.tile([P, SS, 256], bf16, tag="t2")
                nc.vector.tensor_tensor(out=t2, in0=t1[:, :, 0:256],
                                        in1=t1[:, :, 256:512], op=OP.add)
                t3 = trp.tile([P, SS, 128], bf16, tag="t3")
                nc.vector.tensor_tensor(out=t3, in0=t2[:, :, 0:128],
                                        in1=t2[:, :, 128:256], op=OP.add)
                ss_t = ssp.tile([P, SS, PPT], f32, tag="ss")
                ss_tiles[t] = ss_t
                nc.vector.tensor_tensor(out=ss_t, in0=t3[:, :, 0:PPT],
                                        in1=t3[:, :, PPT:128], op=OP.add)
                if t >= 1:
                    p1_mult_mm(t - 1)
            p1_rec(NSUP - 1)
            p1_mult_mm(NSUP - 1)

            # ---- edge points: normalize + one-hot sums ----
            exsq = tmp.tile([P, D], f32, tag="exsq")
            nc.vector.tensor_tensor(out=exsq, in0=ex_sb, in1=ex_sb, op=OP.mult)
            ess = tmp.tile([P, 1], f32, tag="ess")
            nc.vector.tensor_reduce(out=ess, in_=exsq, axis=AX.X, op=OP.add)
            erc = tmp.tile([P, 1], f32, tag="erc")
            nc.scalar.activation(out=erc, in_=ess, func=AF.Abs_reciprocal_sqrt)
            exh = fin.tile([P, D], bf16, tag="exh")
            nc.vector.tensor_scalar_mul(out=exh, in0=ex_sb, scalar1=erc)
            # transposed edge sums: out[d, l] = sum_p exh[p, d] * eoh[p, l]
            esums_ps = ps2.tile([D, L], f32, tag="small")
            nc.tensor.matmul(out=esums_ps, lhsT=exh, rhs=eoh_sb,
                             start=True, stop=True)

            # ---- centroids (in [D, L] orientation, matching d-major rows) ----
            sums_row = fin.tile([1, FD], f32, tag="sums_row")
            nc.vector.tensor_copy(out=sums_row, in_=sums_ps)
            sumsMatT = fin.tile([D, L], f32, tag="sumsMatT")
            nc.sync.dma_start(
                out=sumsMatT, in_=sums_row.rearrange("a (d l) -> a d l", d=D))
            esums_sb = fin.tile([D, L], f32, tag="esums_sb")
            nc.scalar.copy(out=esums_sb, in_=esums_ps)
            totMatT = fin.tile([D, L], f32, tag="totMatT")
            nc.vector.tensor_tensor(out=totMatT, in0=sumsMatT, in1=esums_sb,
                                    op=OP.add)
            muMatT = fin.tile([D, L], bf16, tag="muMatT")
            nc.vector.tensor_tensor(out=muMatT, in0=totMatT, in1=rc_sb,
                                    op=OP.mult)
            muT_ps = ps2.tile([L, D], f32, tag="small")
            nc.tensor.matmul(out=muT_ps, lhsT=muMatT, rhs=id16_sb,
                             start=True, stop=True)
            muMatb = fin.tile([L, D], bf16, tag="muMatb")
            nc.vector.tensor_copy(out=muMatb, in_=muT_ps)
            muRowb = fin.tile([1, FD], bf16, tag="muRowb")
            nc.sync.dma_start(
                out=muRowb.rearrange("a (d l) -> a d l", d=D), in_=muMatT)
            mubc_ps = ps.tile([P, FD], f32, tag="mubc")
            for h in range(2):
                nc.tensor.matmul(
                    out=mubc_ps[:, h * 512:(h + 1) * 512],
                    lhsT=onescol,
                    rhs=muRowb[:, h * 512:(h + 1) * 512],
                    start=True, stop=True)
            muExp = fin.tile([P, FD], bf16, tag="muExp")
            nc.vector.tensor_copy(out=muExp, in_=mubc_ps)

            # ---- pass 2: pull term ----
            pull_ps = psp.tile([1, L], f32, tag="pull")
            ad_tiles = [None] * NSUP

            def p2_tail(t):
                ad3 = ad_tiles[t].rearrange("p (s f) -> p s f", s=SS)
                u1 = trp.tile([P, SS, 512], bf16, tag="u1")
                nc.vector.tensor_tensor(out=u1, in0=ad3[:, :, 0:512],
                                        in1=ad3[:, :, 512:1024], op=OP.add)
                u2 = trp.tile([P, SS, 256], bf16, tag="u2")
                nc.vector.tensor_tensor(out=u2, in0=u1[:, :, 0:256],
                                        in1=u1[:, :, 256:512], op=OP.add)
                u3 = trp.tile([P, SS, 128], bf16, tag="u3")
                nc.vector.tensor_tensor(out=u3, in0=u2[:, :, 0:128],
                                        in1=u2[:, :, 128:256], op=OP.add)
                d1_t = d1p.tile([P, SS, PPT], f32, tag="d1")
                nc.vector.tensor_tensor(out=d1_t, in0=u3[:, :, 0:PPT],
                                        in1=u3[:, :, PPT:128], op=OP.add)
                pp_t = ppp.tile([P, SS * PPT], bf16, tag="pp")
                nc.scalar.activation(out=pp_t,
                                     in_=d1_t.rearrange("p s j -> p (s j)"),
                                     func=AF.Square, bias=negdv)
                for s in range(SS):
                    nc.tensor.matmul(
                        out=pull_ps,
                        lhsT=pat_sb[:, t * SS + s:t * SS + s + 1],
                        rhs=pp_t[:, s * PPT:(s + 1) * PPT],
                        start=(t == 0 and s == 0),
                        stop=(t == NSUP - 1 and s == SS - 1))

            for t in range(NSUP):
                xh_t = xh_tiles[t]
                diff_t = dfp.tile([P, FDS], bf16, tag="diff")
                nc.vector.tensor_tensor(
                    out=diff_t.rearrange("p (s f) -> p s f", s=SS),
                    in0=xh_t.rearrange("p (s f) -> p s f", s=SS),
                    in1=muExp.unsqueeze(1).to_broadcast([P, SS, FD]),
                    op=OP.subtract)
                # |diff| on ACT, then sum over D via contiguous 2x TT tree
                # (a strided tensor_reduce over d-major costs 1.7ns/elem)
                ad_t = adp.tile([P, FDS], bf16, tag="ad")
                ad_tiles[t] = ad_t
                nc.scalar.activation(out=ad_t, in_=diff_t, func=AF.Abs)
                p2_tail(t)

            # ---- edge pull ----
            medge_ps = ps2.tile([P, D], f32, tag="small")
            nc.tensor.matmul(out=medge_ps, lhsT=eohT_sb, rhs=muMatb,
                             start=True, stop=True)
            ediff = tmp.tile([P, D], bf16, tag="ediff")
            nc.vector.tensor_tensor(out=ediff, in0=exh, in1=medge_ps,
                                    op=OP.subtract)
            ed1 = tmp.tile([P, 1], f32, tag="ed1")
            nc.vector.tensor_reduce(out=ed1, in_=ediff, axis=AX.X, op=OP.add,
                                    apply_absolute_value=True)
            erl = tmp.tile([P, 1], bf16, tag="erl")
            nc.scalar.activation(out=erl, in_=ed1, func=AF.Relu, bias=negdv)
            epp = tmp.tile([P, 1], bf16, tag="epp")
            nc.vector.tensor_tensor(out=epp, in0=erl, in1=erl, op=OP.mult)
            pull2_ps = ps2.tile([1, L], f32, tag="small")
            nc.tensor.matmul(out=pull2_ps, lhsT=epp, rhs=eoh_sb,
                             start=True, stop=True)

            # ---- push term (pairwise centroid distances) ----
            diffP = fin.tile([L, L, D], bf16, tag="diffP")
            nc.vector.tensor_tensor(
                out=diffP,
                in0=muExp[:L, :].rearrange("l (d m) -> l m d", d=D),
                in1=muMatb.unsqueeze(1).to_broadcast([L, L, D]),
                op=OP.subtract)
            distP = fin.tile([L, L], f32, tag="distP")
            nc.vector.tensor_reduce(out=distP, in_=diffP, axis=AX.X, op=OP.add,
                                    apply_absolute_value=True)
            hingeP = fin.tile([L, L], f32, tag="hingeP")
            nc.scalar.activation(out=hingeP, in_=distP, func=AF.Relu,
                                 bias=twodd[:L], scale=-1.0)
            hm = fin.tile([L, L], bf16, tag="hm")
            nc.vector.tensor_tensor(out=hm, in0=hingeP, in1=dm_sb, op=OP.mult)
            psq = fin.tile([L, L], bf16, tag="psq")
            nc.vector.tensor_tensor(out=psq, in0=hm, in1=hm, op=OP.mult)
            pushrow = fin.tile([L, 1], f32, tag="pushrow")
            nc.vector.tensor_reduce(out=pushrow, in_=psq, axis=AX.X, op=OP.add)
            push_ps = ps2.tile([1, 1], f32, tag="small")
            nc.tensor.matmul(out=push_ps, lhsT=pushrow, rhs=ones64,
                             start=True, stop=True)

            # ---- finalize pull ----
            pull_row = fin.tile([1, L], f32, tag="pull_row")
            nc.vector.tensor_copy(out=pull_row, in_=pull_ps)
            pull_tot = fin.tile([1, L], f32, tag="pull_tot")
            nc.vector.tensor_tensor(out=pull_tot, in0=pull_row, in1=pull2_ps,
                                    op=OP.add)
            pullw = fin.tile([1, L], f32, tag="pullw")
            nc.vector.tensor_tensor(out=pullw, in0=pull_tot, in1=rp_sb,
                                    op=OP.mult)
            pullb = fin.tile([1, 1], f32, tag="pullb")
            nc.vector.tensor_reduce(out=pullb, in_=pullw, axis=AX.X, op=OP.add)

            # ---- total ----
            t1f = fin.tile([1, 1], f32, tag="t1f")
            nc.scalar.mul(t1f, push_ps, 1.0 / (L * (L - 1)))
            t2f = fin.tile([1, 1], f32, tag="t2f")
            nc.scalar.add(t2f, t1f, pullb)
            t3f = fin.tile([1, 1], f32, tag="t3f")
            nc.scalar.mul(t3f, t2f, 1.0 / B)
            nc.sync.dma_start(out=out, in_=t3f)

    nc.compile()
    return nc


def _get_program():
    if "nc" not in _PROGRAM_CACHE:
        _PROGRAM_CACHE["nc"] = _build_program()
    return _PROGRAM_CACHE["nc"]


# ----------------------------------------------------------------------------
# host orchestration
# ----------------------------------------------------------------------------
def _prep_core_inputs(x, lab, bounds, b):
    import ml_dtypes
    bf = ml_dtypes.bfloat16

    s, e = int(bounds[b]), int(bounds[b + 1])
    lo = -((-s) // 64) * 64
    hi = (e // 64) * 64
    if hi < lo:
        lo = hi = s  # tiny subbatch: no aligned bulk, everything is an edge
    bulk = hi - lo

    # d-major staging: row r holds x[64r:64r+64, :].T flattened
    xs_pad = np.ones((NSLOT * P, FD), bf)
    if bulk > 0:
        rows_bulk = bulk // 64
        xs_pad[:rows_bulk] = (
            x[lo:hi].reshape(rows_bulk, 64, D).transpose(0, 2, 1)
            .reshape(rows_bulk, FD).astype(bf))

    # pat[partition, slot]: partition p of slot sl covers the 64-point row
    # number sl*128 + p of the bulk; weight 1 iff that row is fully real.
    rows_real = bulk // 64
    row_idx = np.arange(NSLOT * P).reshape(NSLOT, P).T  # [P, NSLOT]
    pat = (row_idx < rows_real).astype(np.float32)

    eidx = np.concatenate([np.arange(s, lo), np.arange(hi, e)])
    ne = len(eidx)
    assert ne <= P
    ex_pad = np.ones((P, D), np.float32)
    eoh = np.zeros((P, L), np.float32)
    if ne > 0:
        ex_pad[:ne] = x[eidx]
        eoh[np.arange(ne), lab[eidx]] = 1.0

    n = e - s
    base = s % 64
    ls = np.arange(L)
    cnt = (n // 64) + (((ls - base) % 64) < (n % 64)).astype(np.int64)

    return {
        "xs": xs_pad,
        "pat": pat.astype(bf),
        "ex": ex_pad,
        "eoh": eoh.astype(bf),
        "eohT": eoh.T.astype(bf).copy(),
        "rc": np.repeat((1.0 / cnt.astype(np.float64))
                        .astype(np.float32)[None, :], D, axis=0),
        "rp": (1.0 / (L * cnt.astype(np.float64))).astype(np.float32)[None, :],
        "dm": (1.0 - np.eye(L, dtype=np.float32)).astype(bf),
        "id16": np.eye(D, dtype=np.float32).astype(bf),
    }


def _check_fast_path(x, lab, sub):
    if x.shape != (N, D):
        return False
    if lab.shape != (N,) or sub.shape != (N,):
        return False
    if not np.array_equal(lab, np.arange(N, dtype=np.int64) % L):
        return False
    if sub.min() < 0 or sub.max() >= B:
        return False
    if np.any(sub[1:] < sub[:-1]):
        return False
    return True


def kernel(outputs, labels, subbatch_indices):
    x = np.asarray(outputs, dtype=np.float32)
    lab = np.asarray(labels).astype(np.int64)
    sub = np.asarray(subbatch_indices).astype(np.int64)

    if not _check_fast_path(x, lab, sub):
        return _reference_numpy(x, lab, sub)

    bounds = np.searchsorted(sub, np.arange(B + 1), side="left")
    sizes = np.diff(bounds)
    if sizes.min() == 0 or sizes.max() > PADPTS + 126:
        return _reference_numpy(x, lab, sub)
    for b in range(B):
        s, e = int(bounds[b]), int(bounds[b + 1])
        lo = -((-s) // 64) * 64
        hi = (e // 64) * 64
        if hi - lo > PADPTS or (e - s) - max(hi - lo, 0) > P:
            return _reference_numpy(x, lab, sub)
        n, base = e - s, s % 64
        cnt = (n // 64) + (((np.arange(L) - base) % 64) < (n % 64))
        if cnt.min() <= 0:
            return _reference_numpy(x, lab, sub)

    from concourse import bass_utils

    nc = _get_program()
    in_maps = [_prep_core_inputs(x, lab, bounds, b) for b in range(B)]
    res = bass_utils.run_bass_kernel_spmd(nc, in_maps, list(range(B)))
    _PROGRAM_CACHE["last_results"] = res
    total = np.float32(0.0)
    for b in range(B):
        total += np.float32(res.results[b]["out"][0, 0])
    return np.float32(total)


if __name__ == "__main__":
    import reference
    inputs = {k: np.asarray(v) for k, v in reference.setup_inputs().items()}
    got = kernel(**inputs)
    print("kernel:", got)
